# revision 13
# baseline (speedup 1.0000x reference)
"""Trainium2 Bass kernel for nn_EndPointSpline.

Reference computation (per batch column b, feature d):
    xt = concat([x0, knots_b, x1])           # [T=128] knot values
    t  = spline_discr[:, b]                  # [T] sorted, t[0]=0, t[-1]=1
    vel[j] = (xt[j+1]-xt[j]) / (t[j+1]-t[j]+1e-10)
    left(q) = searchsorted(t[1:], q, 'left') clipped to [0, T-2]
    y(q) = xt[left] + vel[left] * (q - t[left])

Kernel strategy (data-parallel over B across 8 cores, 16 columns/core):
  Linear interpolation is expressed with P1 hat-function weights so the
  gather becomes ONE K=128 f16 matmul per query tile:
      e1[i,q] = (q - t[i-1]) * r[i-1]     (row 0 uses sentinel -1/1)
      e2[i,q] = (t[i+1] - q) * r[i]       (row 127 uses sentinel 2/1)
      lam[i,q]= relu(min(relu(e1), e2))   -> lerp weights, 2 nonzeros per q
      y[q,d]  = sum_i lam[i,q] * xt[i,d]
  The whole 16-bit datapath (xt in f16, lam in f16, output written f16 and
  upcast to f32 on the host) halves HBM traffic vs f32 and keeps rel-err
  ~1e-3, far inside the 2e-2 gate.  e1/e2 are computed in f32 (query/t
  precision matters for the subtract) and stored f16; overflow to +-inf is
  harmless because for any row at most one of e1/e2 exceeds 1.

  Engine budget per b (the kernel is DVE/ACT-copy bound; NOTE the Pool
  engine must stay compute-idle -- gpsimd tensor_scalar measured ~25us/op
  on HW and pushed the kernel 91us -> 505us):
    DVE  : e1, e2 (2x tensor_scalar), s1=min(relu(e1),e2) fused
           scalar_tensor_tensor, lam=relu(s1) (4x), 1 PSUM evac  (~6.2 us)
    ACT  : 3 PSUM evacs [128,2048] f32->f16 + xf DMA issue       (~5.6 us)
    PE   : 16 f16 matmuls [128x128]@[128x512]                    (~4 us)
  PSUM is 2 tiles x 4 banks; evac chunks convert to f16 into osb and one
  1MiB DMA per (b, group) goes out on the SP HWDGE ring.

  Host-side marshalling: xt is pre-assembled to [B, T, D] f16, and queries
  are permuted within 1024-blocks so each output partition writes an 8KB
  contiguous DRAM run (output lands in ORIGINAL query order; only compute
  order is permuted).
"""

import numpy as np

Q, B, T, D = 2048, 128, 128, 512
NCORES = 8
BL = B // NCORES          # 16 batch columns per core
K = T - 1                 # 127 segments
NQT = Q // 128            # 16 query tiles of 128
GQT = 8                   # query tiles per output DMA group (1MiB transfers)
NG = NQT // GQT           # output groups per b
PGROUP = GQT * 128        # queries per output group (1024)

_PROGRAM = None


def permute_queries(query_t):
    """qperm[g*PGROUP + k*128 + p] = query_t[g*PGROUP + p*GQT + k]."""
    a = np.asarray(query_t, dtype=np.float32).reshape(Q // PGROUP, 128, GQT)
    return np.ascontiguousarray(a.transpose(0, 2, 1).reshape(-1))


def assemble_xt(knots, x0, x1):
    """[B, T, D] f16: rows 0 / 1..T-2 / T-1 = x0 / knots / x1 per column."""
    return np.concatenate(
        [
            np.asarray(x0, dtype=np.float32).transpose(1, 0, 2),
            np.asarray(knots, dtype=np.float32),
            np.asarray(x1, dtype=np.float32).transpose(1, 0, 2),
        ],
        axis=1,
    ).astype(np.float16)


def make_core_inputs(query_t, knots, x0, x1, spline_discr, core):
    """Per-core in_map for the Bass program (applies all host marshalling)."""
    s = slice(core * BL, (core + 1) * BL)
    return {
        "query_t": permute_queries(query_t),
        "xt": np.ascontiguousarray(assemble_xt(knots[s], x0[:, s], x1[:, s])),
        "spline_discr": np.ascontiguousarray(
            np.asarray(spline_discr, dtype=np.float32)[:, s]
        ),
    }


def _build_program(reps=1):
    import concourse.tile as tile
    from concourse import bacc, mybir

    f32 = mybir.dt.float32
    f16 = mybir.dt.float16
    Alu = mybir.AluOpType

    nc = bacc.Bacc("TRN2", target_bir_lowering=False, debug=False)

    q_d = nc.dram_tensor("query_t", [Q], f32, kind="ExternalInput").ap()
    xt_d = nc.dram_tensor("xt", [BL, T, D], f16, kind="ExternalInput").ap()
    t_d = nc.dram_tensor("spline_discr", [T, BL], f32, kind="ExternalInput").ap()
    out_d = nc.dram_tensor("out", [BL, Q, D], f16, kind="ExternalOutput").ap()

    with tile.TileContext(nc) as tc:
        with (
            tc.tile_pool(name="const", bufs=1) as cpool,
            tc.tile_pool(name="xf", bufs=3) as xfpool,
            tc.tile_pool(name="wts", bufs=2) as wpool,
            tc.tile_pool(name="outsb", bufs=3) as outpool,
            tc.tile_pool(name="psum", bufs=2, space="PSUM") as pspool,
        ):
            # --- per-core constants ---
            # qb[p, q] = permuted query_t[q] replicated across 128 partitions
            qb = cpool.tile([T, Q], f32)
            nc.sync.dma_start(out=qb[:], in_=q_d.partition_broadcast(T))
            # r = 1/(dt+1e-10)
            tlo = cpool.tile([K, BL], f32)
            nc.sync.dma_start(out=tlo[:], in_=t_d[0:K, :])
            thi = cpool.tile([K, BL], f32)
            nc.sync.dma_start(out=thi[:], in_=t_d[1:T, :])
            r = cpool.tile([K, BL], f32)
            nc.vector.tensor_tensor(out=r[:], in0=thi[:], in1=tlo[:], op=Alu.subtract)
            nc.vector.tensor_scalar_add(out=r[:], in0=r[:], scalar1=1e-10)
            nc.vector.reciprocal(out=r[:], in_=r[:])
            # hat-function node constants:
            # tA[i]=t[i-1] (row0 -1), rA[i]=r[i-1] (row0 1),
            # tB[i]=t[i+1] (row127 2), nrB[i]=-r[i] (row127 -1)
            tA = cpool.tile([T, BL], f32)
            nc.vector.memset(tA[:], -1.0)
            nc.sync.dma_start(out=tA[1:T, :], in_=t_d[0:K, :])
            rA = cpool.tile([T, BL], f32)
            nc.vector.memset(rA[:], 1.0)
            nc.sync.dma_start(out=rA[1:T, :], in_=r[:])
            tB = cpool.tile([T, BL], f32)
            nc.vector.memset(tB[:], 2.0)
            nc.sync.dma_start(out=tB[0:K, :], in_=t_d[1:T, :])
            nrB = cpool.tile([T, BL], f32)
            nc.vector.memset(nrB[:], -1.0)
            nc.vector.tensor_scalar_mul(out=nrB[0:K, :], in0=r[:], scalar1=-1.0)

            for rep in range(reps):
                for b in range(BL):
                    # xt rows 0..127, one contiguous 128KB DMA (ACT HWDGE)
                    xf = xfpool.tile([T, D], f16)
                    nc.scalar.dma_start(out=xf[:], in_=xt_d[b, :, :])

                    # hat weights over all 2048 queries, all on DVE
                    e1 = wpool.tile([T, Q], f16)
                    nc.vector.tensor_scalar(
                        out=e1[:], in0=qb[:], scalar1=tA[:, b : b + 1],
                        scalar2=rA[:, b : b + 1], op0=Alu.subtract, op1=Alu.mult,
                    )
                    e2 = wpool.tile([T, Q], f16)
                    nc.vector.tensor_scalar(
                        out=e2[:], in0=qb[:], scalar1=tB[:, b : b + 1],
                        scalar2=nrB[:, b : b + 1], op0=Alu.subtract, op1=Alu.mult,
                    )
                    s1 = wpool.tile([T, Q], f16)
                    nc.vector.scalar_tensor_tensor(
                        out=s1[:], in0=e1[:], scalar=0.0, in1=e2[:],
                        op0=Alu.max, op1=Alu.min,
                    )
                    lam = wpool.tile([T, Q], f16)
                    nc.vector.tensor_scalar_max(out=lam[:], in0=s1[:], scalar1=0.0)

                    for g in range(NG):
                        osb = outpool.tile([128, GQT * D], f16)
                        for h in range(GQT // 4):
                            ps = pspool.tile([128, 4 * D], f32)
                            for k2 in range(4):
                                qt = g * GQT + h * 4 + k2
                                sl = slice(qt * 128, (qt + 1) * 128)
                                nc.tensor.matmul(
                                    ps[:, k2 * D : (k2 + 1) * D],
                                    lhsT=lam[:, sl], rhs=xf[:],
                                    start=True, stop=True,
                                )
                            # evacuate 4 PSUM banks -> f16; 1 chunk on DVE,
                            # 3 on ACT per b (GQT=16 single-DMA and DVE/ACT
                            # tail-splits both measured slower; the kernel
                            # sits at the ~435 GB/s DMA roofline, and the
                            # 2-group structure keeps the SP ring drained
                            # while the second group's chunks evacuate)
                            dst = osb[:, h * 4 * D : (h + 1) * 4 * D]
                            if g == 0 and h == 0:
                                nc.vector.tensor_copy(out=dst, in_=ps[:])
                            else:
                                nc.scalar.copy(out=dst, in_=ps[:])
                        # one 1MiB DMA per group; group 0 on the SP HWDGE
                        # ring, group 1 on the idle SWDGE (gpsimd) ring so
                        # the two descriptor rings drain in parallel.  (Using
                        # the ACT HWDGE ring instead measured slower: the
                        # ACT-stream issuance stalls its PSUM copies.)
                        dview = out_d[
                            b, g * PGROUP : (g + 1) * PGROUP, :
                        ].rearrange("(p c) d -> p (c d)", p=128)
                        eng = nc.sync if g == 0 else nc.gpsimd
                        eng.dma_start(out=dview, in_=osb[:])
    nc.finalize()
    return nc


def _get_program(reps=1):
    global _PROGRAM
    if _PROGRAM is None:
        _PROGRAM = {}
    if reps not in _PROGRAM:
        _PROGRAM[reps] = _build_program(reps)
    return _PROGRAM[reps]


def kernel(query_t, knots, x0, x1, spline_discr, _trace=False, **_trace_kwargs):
    from concourse.bass_utils import run_bass_kernel_spmd

    query_t = np.asarray(query_t, dtype=np.float32)
    knots = np.asarray(knots, dtype=np.float32)
    x0 = np.asarray(x0, dtype=np.float32)
    x1 = np.asarray(x1, dtype=np.float32)
    spline_discr = np.asarray(spline_discr, dtype=np.float32)

    nc = _get_program()
    in_maps = [
        make_core_inputs(query_t, knots, x0, x1, spline_discr, c)
        for c in range(NCORES)
    ]
    res = run_bass_kernel_spmd(
        nc, in_maps, core_ids=list(range(NCORES)), trace=_trace, **_trace_kwargs
    )
    out = np.concatenate(
        [np.asarray(r["out"], dtype=np.float32) for r in res.results], axis=0
    )
    if _trace:
        return out, res
    return out
